# revision 17
# baseline (speedup 1.0000x reference)
"""Sliding-window GQA attention (softcap) on 8 trn2 NeuronCores.

Problem shapes (hardcoded):
  Q [1, 32, 2048, 128] bf16, K/V [1, 8, 2048, 128] bf16 -> out [1, 32, 2048, 128] f32
  causal, window_left=256, softcap=30, scale=1/sqrt(128), GQA group=4.

Sharding: core c owns kv-head c and query heads [4c, 4c+4). Each (b, h_kv)
slice is fully independent -> no collectives.

Per-core kernel, v2 (ACT-pipelined):
  ACT (tanh+exp over all 5760 score cols/head, ~11.6us/head busy) is the
  bottleneck engine; the schedule keeps it streaming:
  - a warmup activation at t=0 pulls the ~2.7us ACT table load into the DMA
    lead-in; K^T/Q^T transposes go on two HWDGE rings in parallel.
  - ACT order per head: [tanh g0..g3, exp(strips 0-7), tanh g4..g7,
    exp(strips 8-15)]; the two big exp chunks amortize the ~330-cycle
    per-instruction overhead.
  - software-pipelined carry: head h's second-half mask/PV/normalize/DMA is
    emitted inside head h+1's loop, so on the TensorE queue it lands in the
    exp-A(h+1) window and never delays the QK matmuls that feed tanh.
  - scores stay in the transposed S^T[k, q] layout (strip per key-block) so
    post-softmax P^T is directly the PV lhsT; softcap bounds scores at +-30
    so exp uses the constant shift 30 (no row max).
  - PV accumulates O (+ row-sum via a ones-column in V) into [128,4,256]
    psum quads; normalize is one recip + one broadcast-mul per quad.
  - the last head's tail runs exp/PV/normalize/DMA per strip-pair.
"""

import math
from contextlib import ExitStack

import numpy as np

import concourse.bacc as bacc
import concourse.bass as bass
import concourse.mybir as mybir
import concourse.tile as tile
from concourse.bass import MemorySpace
from concourse.bass_utils import run_bass_kernel_spmd

BF16 = mybir.dt.bfloat16
F32 = mybir.dt.float32

N_CORES = 8
HQ_PER_CORE = 4  # GQA group size
SQ = 2048
D = 128
NB = SQ // 128  # 16 key/query blocks
SCALE = 1.0 / math.sqrt(128.0)
SOFTCAP = 30.0

# strip widths: key-block kb sees q-columns [kb*128, kb*128 + W[kb])
WIDTHS = [min(384, SQ - kb * 128) for kb in range(NB)]
OFFS = [sum(WIDTHS[:kb]) for kb in range(NB)]
TOT = sum(WIDTHS)  # 5760 score columns per head


def build_attention(nc: bass.Bass, q, k, v, mask, out):
    """q [4,128,2048] bf16 (pre-transposed); k [128,2048] bf16 (pre-transposed);
    v [2048,129] bf16 (ones col appended); mask [128,2,128] bf16;
    out [4,2048,128] f32 (DRAM APs)."""
    with ExitStack() as ctx:
        tc = ctx.enter_context(tile.TileContext(nc))
        consts = ctx.enter_context(tc.tile_pool(name="consts", bufs=1))
        qt_pool = ctx.enter_context(tc.tile_pool(name="qt", bufs=3))
        t_pool = ctx.enter_context(tc.tile_pool(name="tbuf", bufs=2))
        p_pool = ctx.enter_context(tc.tile_pool(name="pbuf", bufs=2))
        o_pool = ctx.enter_context(tc.tile_pool(name="obuf", bufs=2))
        r_pool = ctx.enter_context(tc.tile_pool(name="rtile", bufs=4))
        tail_pool = ctx.enter_context(tc.tile_pool(name="tailp", bufs=4))
        spsum = ctx.enter_context(
            tc.tile_pool(name="spsum", bufs=3, space=MemorySpace.PSUM)
        )
        opsum = ctx.enter_context(
            tc.tile_pool(name="opsum", bufs=2, space=MemorySpace.PSUM)
        )

        # ---- t=0: ACT table-load warmup (exp set includes tanh). The memset
        # runs on DVE (gpsimd's first instruction pays a ~6us IRAM load).
        warm = consts.tile([128, 2], F32)
        nc.vector.memset(warm[:, 0:1], 0.0)
        nc.scalar.activation(
            out=warm[:, 1:2], in_=warm[:, 0:1],
            func=mybir.ActivationFunctionType.Exp,
        )
        negcap = consts.tile([128, 1], F32)
        nc.vector.memset(negcap, -SOFTCAP)

        # ---- input staging. Q^T/K^T come pre-transposed from the host and
        # V arrives with the ones-column appended, so every load is a plain
        # wide DMA (no xbar transposes, no gpsimd, no memsets). K^T rides the
        # scalar ring at t=0 (its only trigger); everything else is sync-ring.
        kta = consts.tile([128, 1024], BF16)
        ktb = consts.tile([128, 1024], BF16)
        qts = [
            qt_pool.tile([128, SQ], BF16, name=f"qt{h}", tag="qt")
            for h in range(HQ_PER_CORE)
        ]
        nc.scalar.dma_start(out=kta, in_=k[:, 0:1024])
        nc.scalar.dma_start(out=ktb, in_=k[:, 1024:SQ])
        nc.sync.dma_start(out=qts[0], in_=q[0])
        nc.sync.dma_start(out=qts[1], in_=q[1])

        def kt_blk(kb):
            t, kb = (kta, kb) if kb < 8 else (ktb, kb - 8)
            return t[:, kb * 128 : (kb + 1) * 128]

        def qt_rhs(h, kb, w):
            return qts[h][:, kb * 128 : kb * 128 + w]

        # V blocks + host-appended ones column
        vt = consts.tile([128, NB, 129], BF16)
        nc.sync.dma_start(
            out=vt, in_=v.rearrange("(t p) d -> p t d", p=128)
        )
        # band masks from the host, [128, 2, 128]: slot 0 keeps c >= kr (upper
        # tri incl diag, strip block 0), slot 1 keeps c <= kr (strip block 2)
        muL = consts.tile([128, 2, 128], BF16)
        nc.sync.dma_start(out=muL, in_=mask)

        def qk_group(h, g):
            """Scores for strips (2g, 2g+1) -> one 2-bank psum tile.
            Group 7 (widths 256+128) is packed contiguously so its tanh is a
            single [128, 384] instruction."""
            kb0, kb1 = 2 * g, 2 * g + 1
            sp = spsum.tile([128, 1024], F32, name="sp", tag="sp")
            off1 = WIDTHS[kb0] if g == 7 else 512
            for kb, off in ((kb0, 0), (kb1, off1)):
                w = WIDTHS[kb]
                nc.tensor.matmul(
                    out=sp[:, off : off + w],
                    lhsT=kt_blk(kb),
                    rhs=qt_rhs(h, kb, w),
                    start=True,
                    stop=True,
                )
            return sp

        def tanh_group(g, sp, tbuf):
            kb0, kb1 = 2 * g, 2 * g + 1
            if g == 7:
                w = WIDTHS[kb0] + WIDTHS[kb1]
                nc.scalar.activation(
                    out=tbuf[:, OFFS[kb0] : OFFS[kb0] + w],
                    in_=sp[:, 0:w],
                    func=mybir.ActivationFunctionType.Tanh,
                    scale=SCALE / SOFTCAP,
                )
            else:
                w = WIDTHS[kb0]
                src = sp[:].rearrange("p (g x) -> p g x", g=2)[:, :, 0:w]
                dst = tbuf[:, OFFS[kb0] : OFFS[kb0] + 2 * w].rearrange(
                    "p (g x) -> p g x", g=2
                )
                nc.scalar.activation(
                    out=dst, in_=src,
                    func=mybir.ActivationFunctionType.Tanh,
                    scale=SCALE / SOFTCAP,
                )

        def exp_chunk(tbuf, pbuf, lo, hi):
            nc.scalar.activation(
                out=pbuf[:, lo:hi], in_=tbuf[:, lo:hi],
                func=mybir.ActivationFunctionType.Exp,
                scale=SOFTCAP, bias=negcap,
            )

        def mask_strips(pbuf, kb_lo, kb_hi):
            """Zero invalid triangles of strips [kb_lo, kb_hi)."""
            for kb in range(kb_lo, kb_hi):
                off = OFFS[kb]
                if WIDTHS[kb] == 384:
                    view = pbuf[:, off : off + 384].rearrange(
                        "p (a x) -> p a x", x=128
                    )[:, ::2, :]
                    nc.vector.tensor_mul(out=view, in0=view, in1=muL)
                else:
                    nc.vector.tensor_mul(
                        out=pbuf[:, off : off + 128],
                        in0=pbuf[:, off : off + 128],
                        in1=muL[:, 0, :],
                    )

        def pv_qb(pbuf, otile, qb):
            """Accumulate O[qb] (+ rowsum col 128) into otile slot qb%2."""
            kbs = [kb for kb in (qb - 2, qb - 1, qb) if kb >= 0]
            for kb in kbs:
                j = qb - kb
                nc.tensor.matmul(
                    out=otile[:, qb % 2, 0:129],
                    lhsT=pbuf[:, OFFS[kb] + j * 128 : OFFS[kb] + (j + 1) * 128],
                    rhs=vt[:, kb, :],
                    start=(kb == kbs[0]),
                    stop=(kb == qb),
                )

        def normalize_pair(otile, hs, pair, dma=True):
            """Normalize qb pair (2*pair, 2*pair+1), write obuf, DMA out."""
            obuf, out_v = hs["obuf"], hs["out_v"]
            rt = r_pool.tile([128, 2], F32)
            nc.vector.reciprocal(out=rt, in_=otile[:, :, 128])
            nc.vector.tensor_mul(
                out=obuf[:, 2 * pair : 2 * pair + 2, :],
                in0=otile[:, :, 0:128],
                in1=rt.to_broadcast([128, 2, 128]),
            )
            if dma:
                nc.sync.dma_start(
                    out=out_v[:, 2 * pair : 2 * pair + 2, :],
                    in_=obuf[:, 2 * pair : 2 * pair + 2, :],
                )

        def pv_half(hs, half, one_dma=False):
            """PV/normalize/DMA for strips 8*half..8*half+8 (mask already done)."""
            pbuf = hs["pbuf"]
            for pair in range(4 * half, 4 * half + 4):
                ot = opsum.tile([128, 2, 132], F32, name="ot", tag="ot")
                pv_qb(pbuf, ot, 2 * pair)
                pv_qb(pbuf, ot, 2 * pair + 1)
                normalize_pair(ot, hs, pair, dma=not one_dma)
            if one_dma:
                qb0 = 8 * half
                nc.sync.dma_start(
                    out=hs["out_v"][:, qb0 : qb0 + 8, :],
                    in_=hs["obuf"][:, qb0 : qb0 + 8, :],
                )

        pending = {}
        carry = None  # head state whose second half still needs PV/out
        for h in range(HQ_PER_CORE):
            if h + 2 < HQ_PER_CORE:
                nc.sync.dma_start(out=qts[h + 2], in_=q[h + 2])
            hs = {
                "h": h,
                "tbuf": t_pool.tile([128, TOT], F32, name="tbuf", tag="tbuf"),
                "pbuf": p_pool.tile([128, TOT], BF16, name="pbuf", tag="pbuf"),
                "obuf": o_pool.tile([128, NB, 128], BF16, name="obuf", tag="obuf"),
                "out_v": out[h].rearrange("(qb p) d -> p qb d", p=128),
            }
            # ---- ACT first half: tanh g0..g3, exp(strips 0-7), mask right away
            for g in range(4):
                sp = pending.pop((h, g), None)
                if sp is None:
                    sp = qk_group(h, g)
                tanh_group(g, sp, hs["tbuf"])
            exp_chunk(hs["tbuf"], hs["pbuf"], 0, OFFS[8])
            mask_strips(hs["pbuf"], 0, 8)
            # TE work for the exp-A window: previous head's second-half PV
            if carry is not None:
                pv_half(carry, 1)
                carry = None
            # ---- ACT second half: tanh g4..g7, exp(strips 8-15)
            for g in range(4, 8):
                tanh_group(g, qk_group(h, g), hs["tbuf"])
            # hoist next head's first two QK groups (TE work + early tanh input)
            if h + 1 < HQ_PER_CORE:
                pending[(h + 1, 0)] = qk_group(h + 1, 0)
                pending[(h + 1, 1)] = qk_group(h + 1, 1)
            if h < HQ_PER_CORE - 1:
                exp_chunk(hs["tbuf"], hs["pbuf"], OFFS[8], TOT)
                mask_strips(hs["pbuf"], 8, NB)
                # TE work for the exp-B window: this head's first-half PV
                pv_half(hs, 0)
                carry = hs
            else:
                # last head: first half, then a fine-grained tail per strip-pair
                pv_half(hs, 0, one_dma=True)
                pbuf, obuf, out_v = hs["pbuf"], hs["obuf"], hs["out_v"]
                deferred = []
                tails = [
                    tail_pool.tile([128, OFFS[2 * g + 1] + WIDTHS[2 * g + 1]
                                    - OFFS[2 * g]], BF16,
                                   name=f"ptail{g}", tag=f"ptail{g}")
                    for g in range(4, 8)
                ]
                for g in range(4, 8):
                    kb0, kb1 = 2 * g, 2 * g + 1
                    pt = tails[g - 4]
                    base = OFFS[kb0]
                    nc.scalar.activation(
                        out=pt, in_=hs["tbuf"][:, base : OFFS[kb1] + WIDTHS[kb1]],
                        func=mybir.ActivationFunctionType.Exp,
                        scale=SOFTCAP, bias=negcap,
                    )
                    for kb in (kb0, kb1):
                        off = OFFS[kb] - base
                        if WIDTHS[kb] == 384:
                            view = pt[:, off : off + 384].rearrange(
                                "p (a x) -> p a x", x=128
                            )[:, ::2, :]
                            nc.vector.tensor_mul(out=view, in0=view, in1=muL)
                        else:
                            nc.vector.tensor_mul(
                                out=pt[:, off : off + 128],
                                in0=pt[:, off : off + 128],
                                in1=muL[:, 0, :],
                            )
                    ot = opsum.tile([128, 2, 132], F32, name="ot", tag="ot")
                    for qb in (kb0, kb1):
                        kbs = [kb for kb in (qb - 2, qb - 1, qb) if kb >= 0]
                        for kb in kbs:
                            j = qb - kb
                            if kb >= 8:
                                gsrc = tails[(kb - 8) // 2]
                                lo = OFFS[kb] - OFFS[2 * (kb // 2)] + j * 128
                            else:
                                gsrc = pbuf
                                lo = OFFS[kb] + j * 128
                            nc.tensor.matmul(
                                out=ot[:, qb % 2, 0:129],
                                lhsT=gsrc[:, lo : lo + 128],
                                rhs=vt[:, kb, :],
                                start=(kb == kbs[0]),
                                stop=(kb == qb),
                            )
                    rt = r_pool.tile([128, 2], F32)
                    nc.vector.reciprocal(out=rt, in_=ot[:, :, 128])
                    nc.vector.tensor_mul(
                        out=obuf[:, kb0 : kb0 + 2, :],
                        in0=ot[:, :, 0:128],
                        in1=rt.to_broadcast([128, 2, 128]),
                    )
                    # final pieces ride the now-idle scalar ring; their
                    # triggers are emitted only after all ACT work so they
                    # never block an activation in the FIFO
                    deferred.append(kb0)
                for lo in (8, 12):
                    nc.scalar.dma_start(
                        out=out_v[:, lo : lo + 4, :],
                        in_=obuf[:, lo : lo + 4, :],
                    )
    return nc


_CACHED = None


def _build():
    global _CACHED
    if _CACHED is None:
        nc = bacc.Bacc()
        q = nc.dram_tensor("q", [HQ_PER_CORE, D, SQ], BF16, kind="ExternalInput")
        k = nc.dram_tensor("k", [D, SQ], BF16, kind="ExternalInput")
        v = nc.dram_tensor("v", [SQ, D + 1], BF16, kind="ExternalInput")
        mask = nc.dram_tensor("mask", [128, 2, 128], BF16, kind="ExternalInput")
        out = nc.dram_tensor("out", [HQ_PER_CORE, SQ, D], BF16, kind="ExternalOutput")
        build_attention(nc, q[:], k[:], v[:], mask[:], out[:])
        nc.finalize()
        _CACHED = nc
    return _CACHED


def make_in_maps(Q, K, V):
    import ml_dtypes

    Qt = np.asarray(Q).astype(ml_dtypes.bfloat16).reshape(32, SQ, D)
    Qt = np.ascontiguousarray(Qt.transpose(0, 2, 1))  # [32, 128, 2048]
    Kt = np.asarray(K).astype(ml_dtypes.bfloat16).reshape(8, SQ, D)
    Kt = np.ascontiguousarray(Kt.transpose(0, 2, 1))  # [8, 128, 2048]
    Vn = np.asarray(V).astype(ml_dtypes.bfloat16).reshape(8, SQ, D)
    Va = np.concatenate(
        [Vn, np.ones((8, SQ, 1), dtype=ml_dtypes.bfloat16)], axis=2
    )  # [8, 2048, 129]
    r = np.arange(128)
    muL = np.zeros((128, 2, 128), dtype=ml_dtypes.bfloat16)
    muL[:, 0, :] = (r[None, :] >= r[:, None])  # strip block 0: keep c >= kr
    muL[:, 1, :] = (r[None, :] <= r[:, None])  # strip block 2: keep c <= kr
    return [
        {
            "q": np.ascontiguousarray(Qt[4 * c : 4 * c + 4]),
            "k": np.ascontiguousarray(Kt[c]),
            "v": np.ascontiguousarray(Va[c]),
            "mask": muL,
        }
        for c in range(N_CORES)
    ]


def kernel(Q, K, V):
    nc = _build()
    in_maps = make_in_maps(Q, K, V)
    res = run_bass_kernel_spmd(nc, in_maps, list(range(N_CORES))).results
    out = np.stack([np.asarray(res[c]["out"]) for c in range(N_CORES)])
    return out.reshape(1, 32, SQ, D).astype(np.float32)


# revision 18
# speedup vs baseline: 1.0244x; 1.0244x over previous
"""Sliding-window GQA attention (softcap) on 8 trn2 NeuronCores.

Problem shapes (hardcoded):
  Q [1, 32, 2048, 128] bf16, K/V [1, 8, 2048, 128] bf16 -> out [1, 32, 2048, 128] f32
  causal, window_left=256, softcap=30, scale=1/sqrt(128), GQA group=4.

Sharding: core c owns kv-head c and query heads [4c, 4c+4). Each (b, h_kv)
slice is fully independent -> no collectives.

Per-core kernel, v2 (ACT-pipelined):
  ACT (tanh+exp over all 5760 score cols/head, ~11.6us/head busy) is the
  bottleneck engine; the schedule keeps it streaming:
  - a warmup activation at t=0 pulls the ~2.7us ACT table load into the DMA
    lead-in; K^T/Q^T transposes go on two HWDGE rings in parallel.
  - ACT order per head: [tanh g0..g3, exp(strips 0-7), tanh g4..g7,
    exp(strips 8-15)]; the two big exp chunks amortize the ~330-cycle
    per-instruction overhead.
  - software-pipelined carry: head h's second-half mask/PV/normalize/DMA is
    emitted inside head h+1's loop, so on the TensorE queue it lands in the
    exp-A(h+1) window and never delays the QK matmuls that feed tanh.
  - scores stay in the transposed S^T[k, q] layout (strip per key-block) so
    post-softmax P^T is directly the PV lhsT; softcap bounds scores at +-30
    so exp uses the constant shift 30 (no row max).
  - PV accumulates O (+ row-sum via a ones-column in V) into [128,4,256]
    psum quads; normalize is one recip + one broadcast-mul per quad.
  - the last head's tail runs exp/PV/normalize/DMA per strip-pair.
"""

import math
from contextlib import ExitStack

import numpy as np

import concourse.bacc as bacc
import concourse.bass as bass
import concourse.mybir as mybir
import concourse.tile as tile
from concourse.bass import MemorySpace
from concourse.bass_utils import run_bass_kernel_spmd

BF16 = mybir.dt.bfloat16
F32 = mybir.dt.float32

N_CORES = 8
HQ_PER_CORE = 4  # GQA group size
SQ = 2048
D = 128
NB = SQ // 128  # 16 key/query blocks
SCALE = 1.0 / math.sqrt(128.0)
SOFTCAP = 30.0

# strip widths: key-block kb sees q-columns [kb*128, kb*128 + W[kb])
WIDTHS = [min(384, SQ - kb * 128) for kb in range(NB)]
OFFS = [sum(WIDTHS[:kb]) for kb in range(NB)]
TOT = sum(WIDTHS)  # 5760 score columns per head


def build_attention(nc: bass.Bass, q, k, v, mask, out):
    """q [4,128,2048] bf16 (pre-transposed); k [128,2048] bf16 (pre-transposed);
    v [2048,129] bf16 (ones col appended); mask [128,2,128] bf16;
    out [4,2048,128] f32 (DRAM APs)."""
    with ExitStack() as ctx:
        tc = ctx.enter_context(tile.TileContext(nc))
        consts = ctx.enter_context(tc.tile_pool(name="consts", bufs=1))
        qt_pool = ctx.enter_context(tc.tile_pool(name="qt", bufs=3))
        t_pool = ctx.enter_context(tc.tile_pool(name="tbuf", bufs=2))
        p_pool = ctx.enter_context(tc.tile_pool(name="pbuf", bufs=2))
        o_pool = ctx.enter_context(tc.tile_pool(name="obuf", bufs=2))
        r_pool = ctx.enter_context(tc.tile_pool(name="rtile", bufs=4))
        tail_pool = ctx.enter_context(tc.tile_pool(name="tailp", bufs=4))
        spsum = ctx.enter_context(
            tc.tile_pool(name="spsum", bufs=3, space=MemorySpace.PSUM)
        )
        opsum = ctx.enter_context(
            tc.tile_pool(name="opsum", bufs=2, space=MemorySpace.PSUM)
        )

        # ---- t=0: ACT table-load warmup (exp set includes tanh). The memset
        # runs on DVE (gpsimd's first instruction pays a ~6us IRAM load).
        warm = consts.tile([128, 2], F32)
        nc.vector.memset(warm[:, 0:1], 0.0)
        nc.scalar.activation(
            out=warm[:, 1:2], in_=warm[:, 0:1],
            func=mybir.ActivationFunctionType.Exp,
        )
        negcap = consts.tile([128, 1], F32)
        nc.vector.memset(negcap, -SOFTCAP)

        # ---- input staging. Q^T/K^T come pre-transposed from the host and
        # V arrives with the ones-column appended, so every load is a plain
        # wide DMA (no xbar transposes, no gpsimd, no memsets). K^T rides the
        # scalar ring at t=0 (its only trigger); everything else is sync-ring.
        kta = consts.tile([128, 1024], BF16)
        ktb = consts.tile([128, 1024], BF16)
        qts = [
            qt_pool.tile([128, SQ], BF16, name=f"qt{h}", tag="qt")
            for h in range(HQ_PER_CORE)
        ]
        nc.scalar.dma_start(out=kta, in_=k[:, 0:1024])
        nc.scalar.dma_start(out=ktb, in_=k[:, 1024:SQ])
        nc.sync.dma_start(out=qts[0], in_=q[0])
        nc.scalar.dma_start(out=qts[1], in_=q[1])

        def kt_blk(kb):
            t, kb = (kta, kb) if kb < 8 else (ktb, kb - 8)
            return t[:, kb * 128 : (kb + 1) * 128]

        def qt_rhs(h, kb, w):
            return qts[h][:, kb * 128 : kb * 128 + w]

        # V blocks + host-appended ones column
        vt = consts.tile([128, NB, 129], BF16)
        nc.sync.dma_start(
            out=vt, in_=v.rearrange("(t p) d -> p t d", p=128)
        )
        # band masks from the host, [128, 2, 128]: slot 0 keeps c >= kr (upper
        # tri incl diag, strip block 0), slot 1 keeps c <= kr (strip block 2)
        muL = consts.tile([128, 2, 128], BF16)
        nc.sync.dma_start(out=muL, in_=mask)

        def qk_group(h, g):
            """Scores for strips (2g, 2g+1) -> one 2-bank psum tile.
            Group 7 (widths 256+128) is packed contiguously so its tanh is a
            single [128, 384] instruction."""
            kb0, kb1 = 2 * g, 2 * g + 1
            sp = spsum.tile([128, 1024], F32, name="sp", tag="sp")
            off1 = WIDTHS[kb0] if g == 7 else 512
            for kb, off in ((kb0, 0), (kb1, off1)):
                w = WIDTHS[kb]
                nc.tensor.matmul(
                    out=sp[:, off : off + w],
                    lhsT=kt_blk(kb),
                    rhs=qt_rhs(h, kb, w),
                    start=True,
                    stop=True,
                )
            return sp

        def tanh_group(g, sp, tbuf):
            kb0, kb1 = 2 * g, 2 * g + 1
            if g == 7:
                w = WIDTHS[kb0] + WIDTHS[kb1]
                nc.scalar.activation(
                    out=tbuf[:, OFFS[kb0] : OFFS[kb0] + w],
                    in_=sp[:, 0:w],
                    func=mybir.ActivationFunctionType.Tanh,
                    scale=SCALE / SOFTCAP,
                )
            else:
                w = WIDTHS[kb0]
                src = sp[:].rearrange("p (g x) -> p g x", g=2)[:, :, 0:w]
                dst = tbuf[:, OFFS[kb0] : OFFS[kb0] + 2 * w].rearrange(
                    "p (g x) -> p g x", g=2
                )
                nc.scalar.activation(
                    out=dst, in_=src,
                    func=mybir.ActivationFunctionType.Tanh,
                    scale=SCALE / SOFTCAP,
                )

        def exp_chunk(tbuf, pbuf, lo, hi):
            nc.scalar.activation(
                out=pbuf[:, lo:hi], in_=tbuf[:, lo:hi],
                func=mybir.ActivationFunctionType.Exp,
                scale=SOFTCAP, bias=negcap,
            )

        def mask_strips(pbuf, kb_lo, kb_hi):
            """Zero invalid triangles of strips [kb_lo, kb_hi)."""
            for kb in range(kb_lo, kb_hi):
                off = OFFS[kb]
                if WIDTHS[kb] == 384:
                    view = pbuf[:, off : off + 384].rearrange(
                        "p (a x) -> p a x", x=128
                    )[:, ::2, :]
                    nc.vector.tensor_mul(out=view, in0=view, in1=muL)
                else:
                    nc.vector.tensor_mul(
                        out=pbuf[:, off : off + 128],
                        in0=pbuf[:, off : off + 128],
                        in1=muL[:, 0, :],
                    )

        def pv_qb(pbuf, otile, qb):
            """Accumulate O[qb] (+ rowsum col 128) into otile slot qb%2."""
            kbs = [kb for kb in (qb - 2, qb - 1, qb) if kb >= 0]
            for kb in kbs:
                j = qb - kb
                nc.tensor.matmul(
                    out=otile[:, qb % 2, 0:129],
                    lhsT=pbuf[:, OFFS[kb] + j * 128 : OFFS[kb] + (j + 1) * 128],
                    rhs=vt[:, kb, :],
                    start=(kb == kbs[0]),
                    stop=(kb == qb),
                )

        def normalize_pair(otile, hs, pair, dma=True):
            """Normalize qb pair (2*pair, 2*pair+1), write obuf, DMA out."""
            obuf, out_v = hs["obuf"], hs["out_v"]
            rt = r_pool.tile([128, 2], F32)
            nc.vector.reciprocal(out=rt, in_=otile[:, :, 128])
            nc.vector.tensor_mul(
                out=obuf[:, 2 * pair : 2 * pair + 2, :],
                in0=otile[:, :, 0:128],
                in1=rt.to_broadcast([128, 2, 128]),
            )
            if dma:
                nc.sync.dma_start(
                    out=out_v[:, 2 * pair : 2 * pair + 2, :],
                    in_=obuf[:, 2 * pair : 2 * pair + 2, :],
                )

        def pv_half(hs, half, one_dma=False):
            """PV/normalize/DMA for strips 8*half..8*half+8 (mask already done)."""
            pbuf = hs["pbuf"]
            for pair in range(4 * half, 4 * half + 4):
                ot = opsum.tile([128, 2, 132], F32, name="ot", tag="ot")
                pv_qb(pbuf, ot, 2 * pair)
                pv_qb(pbuf, ot, 2 * pair + 1)
                normalize_pair(ot, hs, pair, dma=not one_dma)
            if one_dma:
                qb0 = 8 * half
                nc.sync.dma_start(
                    out=hs["out_v"][:, qb0 : qb0 + 8, :],
                    in_=hs["obuf"][:, qb0 : qb0 + 8, :],
                )

        pending = {}
        carry = None  # head state whose second half still needs PV/out
        for h in range(HQ_PER_CORE):
            if h + 2 < HQ_PER_CORE:
                nc.sync.dma_start(out=qts[h + 2], in_=q[h + 2])
            hs = {
                "h": h,
                "tbuf": t_pool.tile([128, TOT], F32, name="tbuf", tag="tbuf"),
                "pbuf": p_pool.tile([128, TOT], BF16, name="pbuf", tag="pbuf"),
                "obuf": o_pool.tile([128, NB, 128], BF16, name="obuf", tag="obuf"),
                "out_v": out[h].rearrange("(qb p) d -> p qb d", p=128),
            }
            # ---- ACT first half: tanh g0..g3, exp(strips 0-7), mask right away
            for g in range(4):
                sp = pending.pop((h, g), None)
                if sp is None:
                    sp = qk_group(h, g)
                tanh_group(g, sp, hs["tbuf"])
            exp_chunk(hs["tbuf"], hs["pbuf"], 0, OFFS[8])
            mask_strips(hs["pbuf"], 0, 8)
            # TE work for the exp-A window: previous head's second-half PV
            if carry is not None:
                pv_half(carry, 1)
                carry = None
            # ---- ACT second half: tanh g4..g7, exp(strips 8-15)
            for g in range(4, 8):
                tanh_group(g, qk_group(h, g), hs["tbuf"])
            # hoist next head's first two QK groups (TE work + early tanh input)
            if h + 1 < HQ_PER_CORE:
                pending[(h + 1, 0)] = qk_group(h + 1, 0)
                pending[(h + 1, 1)] = qk_group(h + 1, 1)
            if h < HQ_PER_CORE - 1:
                exp_chunk(hs["tbuf"], hs["pbuf"], OFFS[8], TOT)
                mask_strips(hs["pbuf"], 8, NB)
                # TE work for the exp-B window: this head's first-half PV
                pv_half(hs, 0)
                carry = hs
            else:
                # last head: first half, then a fine-grained tail per strip-pair
                pv_half(hs, 0)
                pbuf, obuf, out_v = hs["pbuf"], hs["obuf"], hs["out_v"]
                deferred = []
                tails = [
                    tail_pool.tile([128, OFFS[2 * g + 1] + WIDTHS[2 * g + 1]
                                    - OFFS[2 * g]], BF16,
                                   name=f"ptail{g}", tag=f"ptail{g}")
                    for g in range(4, 8)
                ]
                for g in range(4, 8):
                    kb0, kb1 = 2 * g, 2 * g + 1
                    pt = tails[g - 4]
                    base = OFFS[kb0]
                    nc.scalar.activation(
                        out=pt, in_=hs["tbuf"][:, base : OFFS[kb1] + WIDTHS[kb1]],
                        func=mybir.ActivationFunctionType.Exp,
                        scale=SOFTCAP, bias=negcap,
                    )
                    for kb in (kb0, kb1):
                        off = OFFS[kb] - base
                        if WIDTHS[kb] == 384:
                            view = pt[:, off : off + 384].rearrange(
                                "p (a x) -> p a x", x=128
                            )[:, ::2, :]
                            nc.vector.tensor_mul(out=view, in0=view, in1=muL)
                        else:
                            nc.vector.tensor_mul(
                                out=pt[:, off : off + 128],
                                in0=pt[:, off : off + 128],
                                in1=muL[:, 0, :],
                            )
                    ot = opsum.tile([128, 2, 132], F32, name="ot", tag="ot")
                    for qb in (kb0, kb1):
                        kbs = [kb for kb in (qb - 2, qb - 1, qb) if kb >= 0]
                        for kb in kbs:
                            j = qb - kb
                            if kb >= 8:
                                gsrc = tails[(kb - 8) // 2]
                                lo = OFFS[kb] - OFFS[2 * (kb // 2)] + j * 128
                            else:
                                gsrc = pbuf
                                lo = OFFS[kb] + j * 128
                            nc.tensor.matmul(
                                out=ot[:, qb % 2, 0:129],
                                lhsT=gsrc[:, lo : lo + 128],
                                rhs=vt[:, kb, :],
                                start=(kb == kbs[0]),
                                stop=(kb == qb),
                            )
                    rt = r_pool.tile([128, 2], F32)
                    nc.vector.reciprocal(out=rt, in_=ot[:, :, 128])
                    nc.vector.tensor_mul(
                        out=obuf[:, kb0 : kb0 + 2, :],
                        in0=ot[:, :, 0:128],
                        in1=rt.to_broadcast([128, 2, 128]),
                    )
                    # final pieces ride the now-idle scalar ring; their
                    # triggers are emitted only after all ACT work so they
                    # never block an activation in the FIFO
                    deferred.append(kb0)
                for kb0 in deferred:
                    nc.scalar.dma_start(
                        out=out_v[:, kb0 : kb0 + 2, :],
                        in_=obuf[:, kb0 : kb0 + 2, :],
                    )
    return nc


_CACHED = None


def _build():
    global _CACHED
    if _CACHED is None:
        nc = bacc.Bacc()
        q = nc.dram_tensor("q", [HQ_PER_CORE, D, SQ], BF16, kind="ExternalInput")
        k = nc.dram_tensor("k", [D, SQ], BF16, kind="ExternalInput")
        v = nc.dram_tensor("v", [SQ, D + 1], BF16, kind="ExternalInput")
        mask = nc.dram_tensor("mask", [128, 2, 128], BF16, kind="ExternalInput")
        out = nc.dram_tensor("out", [HQ_PER_CORE, SQ, D], BF16, kind="ExternalOutput")
        build_attention(nc, q[:], k[:], v[:], mask[:], out[:])
        nc.finalize()
        _CACHED = nc
    return _CACHED


def make_in_maps(Q, K, V):
    import ml_dtypes

    Qt = np.asarray(Q).astype(ml_dtypes.bfloat16).reshape(32, SQ, D)
    Qt = np.ascontiguousarray(Qt.transpose(0, 2, 1))  # [32, 128, 2048]
    Kt = np.asarray(K).astype(ml_dtypes.bfloat16).reshape(8, SQ, D)
    Kt = np.ascontiguousarray(Kt.transpose(0, 2, 1))  # [8, 128, 2048]
    Vn = np.asarray(V).astype(ml_dtypes.bfloat16).reshape(8, SQ, D)
    Va = np.concatenate(
        [Vn, np.ones((8, SQ, 1), dtype=ml_dtypes.bfloat16)], axis=2
    )  # [8, 2048, 129]
    r = np.arange(128)
    muL = np.zeros((128, 2, 128), dtype=ml_dtypes.bfloat16)
    muL[:, 0, :] = (r[None, :] >= r[:, None])  # strip block 0: keep c >= kr
    muL[:, 1, :] = (r[None, :] <= r[:, None])  # strip block 2: keep c <= kr
    return [
        {
            "q": np.ascontiguousarray(Qt[4 * c : 4 * c + 4]),
            "k": np.ascontiguousarray(Kt[c]),
            "v": np.ascontiguousarray(Va[c]),
            "mask": muL,
        }
        for c in range(N_CORES)
    ]


def kernel(Q, K, V):
    nc = _build()
    in_maps = make_in_maps(Q, K, V)
    res = run_bass_kernel_spmd(nc, in_maps, list(range(N_CORES))).results
    out = np.stack([np.asarray(res[c]["out"]) for c in range(N_CORES)])
    return out.reshape(1, 32, SQ, D).astype(np.float32)
